# revision 13
# baseline (speedup 1.0000x reference)
"""CoAttention forward on 8 TRN2 NeuronCores.

Data-parallel over batch B=64 (8 batches/core). Re-associated affinity:
C = tanh(Q @ (W_b @ V^T)) — 3.1e8 MACs/batch for the C-chain instead of
the reference association's 6.4e8. All heavy matmuls run as f16 hi/lo
3-pass (exact products on the PE; ~22-bit): P = W_b V^T (pair-packed,
two batches side by side in the 392-wide moving operand), C = Q P,
WqQT = Q W_q^T [NQ,D], WvVT = V W_v^T [NV,D]. The H matmuls run 2-pass
against single-f16 C (baseline-proven noise level):
H_vT = tanh(WvVT + C^T WqQT) [NV,D], H_qT = tanh(WqQT + CT^T WvVT) [NQ,D],
with the tanh addends pre-written into PSUM by the Scalar engine (the PE
is at its 1-col/cycle column roofline, so addend columns are moved off
it); the products then accumulate with start=False. CT comes from DMA
xbar transposes of (128-padded) C tiles rather than PE transposes, and a
post-legalize pass drops Ldweights whose stationary matches the previous
load (PE weights persist across matmuls).
(float32r was tried for the downstream matmuls — 3x fewer PE rows — but
its FP22 product rounding leaves 3e-3-rms noise on WqQT/WvVT which the
sqrt(512)-amplifying H matmuls turn into ~0.7 logit noise: tail batches
flip a top softmax logit and blow the 2e-2 gate.)
H stays [token, D] so the w_h dots run on the Vector engine; the
per-partition logit columns are PE-transposed into [1, n] psum rows for
the softmax + on-chip f16 broadcast. v_hat/q_hat are DVE dots against
the f16-hi inputs. Batch-0 inputs are DMA'd ahead of the big weight
tensors so the PE starts early.

kernel(**inputs) takes FULL inputs, shards internally, returns (v_hat, q_hat).
"""
import numpy as np

import concourse.bass as bass
import concourse.mybir as mybir
import concourse.tile as tile
from concourse import bacc
from concourse.bass_utils import run_bass_kernel_spmd
from concourse.masks import make_identity

AF = mybir.ActivationFunctionType
ALU = mybir.AluOpType
AX = mybir.AxisListType
F32 = mybir.dt.float32
F16 = mybir.dt.float16


def _dedup_ldweights(nc):
    """Drop Ldweights whose stationary AP matches the immediately preceding
    weight load with no intervening self-loading matmul: PE weights persist
    across matmuls (verified on HW), so the reload is pure overhead (~95ns
    per 128-row load, ~1519 elidable per core = ~136us). Ldweights carry no
    semaphore updates, so deletion is sem-safe; any waits are merged into
    the next instruction (compile() later re-legalizes multi-waits).
    Must run after TileContext exit (legalize done) and before finalize().
    """
    def sig(inst):
        ap = inst.ins[0]
        return (ap.memref, ap.offset, str(ap.ap), str(ap.dtype),
                str(inst.tile_size), str(inst.tile_position),
                str(inst.perf_mode), str(inst.is_transpose))

    ndrop = 0
    for b in nc.m.functions[0].blocks:
        insts = list(b.instructions)
        cur = None
        keep = []
        for inst in insts:
            if inst.opcode == 'Ldweights':
                si = inst.sync_info
                clean = si is None or (not len(si.on_wait)
                                       and not len(si.on_update))
                s = sig(inst)
                if s == cur and clean:
                    ndrop += 1
                    continue
                cur = s
            elif inst.opcode == 'Matmult' and inst.ldweights is None:
                cur = None  # self-loading matmul clobbers PE weights
            keep.append(inst)
        if len(keep) != len(insts):
            b.instructions[:] = keep
    return ndrop

B, NV, NQ, D = 64, 196, 512, 1024
NCORES = 8
NB = B // NCORES          # batches per core
KD = D // 128             # 8 feature k-tiles
MQ = NQ // 128            # 4 NQ tiles
NV1 = NV - 128            # 68 (second NV chunk)
VROWS = (128, NV1)


def build(nb=NB, dbg=False):
    nc = bacc.Bacc(None, target_bir_lowering=False)

    QTh_d = nc.dram_tensor("QTh", [nb, D, NQ], F16, kind="ExternalInput")
    QTl_d = nc.dram_tensor("QTl", [nb, D, NQ], F16, kind="ExternalInput")
    VTh_d = nc.dram_tensor("VTh", [nb, D, NV], F16, kind="ExternalInput")
    VTl_d = nc.dram_tensor("VTl", [nb, D, NV], F16, kind="ExternalInput")
    WbTh_d = nc.dram_tensor("WbTh", [D, D], F16, kind="ExternalInput")
    WbTl_d = nc.dram_tensor("WbTl", [D, D], F16, kind="ExternalInput")
    WqTh_d = nc.dram_tensor("WqTh", [D, D], F16, kind="ExternalInput")
    WqTl_d = nc.dram_tensor("WqTl", [D, D], F16, kind="ExternalInput")
    WvTh_d = nc.dram_tensor("WvTh", [D, D], F16, kind="ExternalInput")
    WvTl_d = nc.dram_tensor("WvTl", [D, D], F16, kind="ExternalInput")
    wvb_d = nc.dram_tensor("wvb", [128, D], F32, kind="ExternalInput")
    wqb_d = nc.dram_tensor("wqb", [128, D], F32, kind="ExternalInput")
    OV_d = nc.dram_tensor("OV", [nb, D], F32, kind="ExternalOutput")
    OQ_d = nc.dram_tensor("OQ", [nb, D], F32, kind="ExternalOutput")
    DBG = {}
    if dbg:
        for nm, shp in [("dHV", [128, 2]), ("dHQ", [128, MQ])]:
            DBG[nm] = nc.dram_tensor(nm, shp, F32, kind="ExternalOutput")
        for nm, shp in [("dCh", [128, MQ, NV]), ("dCl", [128, MQ, NV]),
                        ("dWQh", [128, MQ, D]), ("dWQl", [128, MQ, D]),
                        ("dWVh", [128, 2, D]), ("dWVl", [128, 2, D]),
                        ("dHT", [128, D]), ("dPh", [128, KD, NV]),
                        ("dPl", [128, KD, NV])]:
            DBG[nm] = nc.dram_tensor(nm, shp, F16, kind="ExternalOutput")

    with tile.TileContext(nc) as tc:
        with (
            tc.tile_pool(name="wsb", bufs=1) as wsb,
            tc.tile_pool(name="qthp", bufs=2) as qthp,
            tc.tile_pool(name="vthp", bufs=2) as vthp,
            tc.tile_pool(name="io1", bufs=1) as io1,
            tc.tile_pool(name="mid", bufs=1) as mid,
            tc.tile_pool(name="hp", bufs=2) as hp,
            tc.tile_pool(name="sm", bufs=1) as sm,
            tc.tile_pool(name="psC", bufs=4, space="PSUM") as psC,
            tc.tile_pool(name="ps5", bufs=2, space="PSUM") as ps5,
            # logit rows live across whole H phases; a dedicated pool keeps
            # their long accumulation groups from pinning ps5's matmul slots
            tc.tile_pool(name="psR", bufs=1, space="PSUM") as psR,
        ):
            # ---- persistent weights ----
            wbth2 = []
            wbtl2 = []
            for hi, kh in enumerate((slice(0, 4), slice(4, KD))):
                th = wsb.tile([128, 4, D], F16, name=f"wbth{hi}")
                nc.sync.dma_start(out=th,
                                  in_=WbTh_d.rearrange("(k p) d -> p k d", p=128)[:, kh, :])
                wbth2.append(th)
                tl = wsb.tile([128, 4, D], F16, name=f"wbtl{hi}")
                nc.sync.dma_start(out=tl,
                                  in_=WbTl_d.rearrange("(k p) d -> p k d", p=128)[:, kh, :])
                wbtl2.append(tl)
            # batch-0 inputs first so P can start while big weights stream
            pre_in = {}
            for _b in (0, 1):
                pv = vthp.tile([128, KD, 2, NV], F16, tag="vthp", name=f"vthp{_b}")
                pl_ = vthp.tile([128, KD, 2, NV], F16, tag="vtlp", name=f"vtlp{_b}")
                for kh in ((slice(0, 4), slice(4, KD)) if _b == 0 else (slice(0, KD),)):
                    for t in (0, 1):
                        nc.sync.dma_start(
                            out=pv[:, kh, t, :],
                            in_=VTh_d[2 * _b + t].rearrange("(k p) n -> p k n", p=128)[:, kh, :])
                        nc.sync.dma_start(
                            out=pl_[:, kh, t, :],
                            in_=VTl_d[2 * _b + t].rearrange("(k p) n -> p k n", p=128)[:, kh, :])
                pre_in[_b] = (pv, pl_)
                if _b == 0:
                    q0 = qthp.tile([128, KD, NQ], F16, tag="qth", name="qth0")
                    nc.sync.dma_start(out=q0, in_=QTh_d[0].rearrange("(k p) n -> p k n", p=128))
                    q0l = io1.tile([128, KD, NQ], F16, tag="qtl", name="qtl0")
                    nc.sync.dma_start(out=q0l, in_=QTl_d[0].rearrange("(k p) n -> p k n", p=128))
                    pre_in["q0"] = (q0, q0l)
            wqth = wsb.tile([128, KD, D], F16, name="wqth")
            nc.sync.dma_start(out=wqth, in_=WqTh_d.rearrange("(k p) d -> p k d", p=128))
            wqtl = wsb.tile([128, KD, D], F16, name="wqtl")
            nc.sync.dma_start(out=wqtl, in_=WqTl_d.rearrange("(k p) d -> p k d", p=128))
            wvth = wsb.tile([128, KD, D], F16, name="wvth")
            nc.sync.dma_start(out=wvth, in_=WvTh_d.rearrange("(k p) d -> p k d", p=128))
            wvtl = wsb.tile([128, KD, D], F16, name="wvtl")
            nc.sync.dma_start(out=wvtl, in_=WvTl_d.rearrange("(k p) d -> p k d", p=128))
            wvb = wsb.tile([128, D], F32, name="wvb")
            nc.sync.dma_start(out=wvb, in_=wvb_d[:, :])
            wqb = wsb.tile([128, D], F32, name="wqb")
            nc.sync.dma_start(out=wqb, in_=wqb_d[:, :])
            identf = wsb.tile([128, 128], F32, name="identf")
            make_identity(nc, identf)
            ones_row = wsb.tile([1, 128], F32)
            nc.vector.memset(ones_row, 1.0)

            phl_pair = [None]
            for b in range(nb):
                pi = b % 2
                po = pi * NV
                if pi == 0:
                    pair = b // 2
                    if pair in pre_in:
                        vthp_t, vtlp_t = pre_in.pop(pair)
                    else:
                        vthp_t = vthp.tile([128, KD, 2, NV], F16, tag="vthp",
                                           name=f"vthp{pair}")
                        nc.sync.dma_start(out=vthp_t[:, :, 0, :],
                                          in_=VTh_d[b].rearrange("(k p) n -> p k n", p=128))
                        nc.sync.dma_start(out=vthp_t[:, :, 1, :],
                                          in_=VTh_d[b + 1].rearrange("(k p) n -> p k n", p=128))
                        vtlp_t = vthp.tile([128, KD, 2, NV], F16, tag="vtlp",
                                           name=f"vtlp{pair}")
                        nc.sync.dma_start(out=vtlp_t[:, :, 0, :],
                                          in_=VTl_d[b].rearrange("(k p) n -> p k n", p=128))
                        nc.sync.dma_start(out=vtlp_t[:, :, 1, :],
                                          in_=VTl_d[b + 1].rearrange("(k p) n -> p k n", p=128))
                vth = vthp_t.rearrange("p k t n -> p k (t n)")[:, :, po:po + NV]
                vtl = vtlp_t.rearrange("p k t n -> p k (t n)")[:, :, po:po + NV]
                if b == 0 and "q0" in pre_in:
                    qth, qtl = pre_in.pop("q0")
                else:
                    qth = qthp.tile([128, KD, NQ], F16, tag="qth", name=f"qth{b}")
                    nc.sync.dma_start(out=qth, in_=QTh_d[b].rearrange("(k p) n -> p k n", p=128))
                    qtl = io1.tile([128, KD, NQ], F16, tag="qtl", name=f"qtl{b}")
                    nc.sync.dma_start(out=qtl, in_=QTl_d[b].rearrange("(k p) n -> p k n", p=128))


                # ---- P = W_b V^T  [D, 2*NV]  f16 3-pass, pair-packed ----
                if pi == 0:
                    vhp2 = vthp_t.rearrange("p k t n -> p k (t n)")
                    vlp2 = vtlp_t.rearrange("p k t n -> p k (t n)")
                    php = mid.tile([128, KD, 2 * NV], F16, tag="php")
                    plp = mid.tile([128, KD, 2 * NV], F16, tag="plp")
                    for dt in range(KD):
                        ds = slice(dt * 128, (dt + 1) * 128)
                        pp = psC.tile([128, 2 * NV], F32, tag="pc", name=f"pp{b}_{dt}")
                        n = 0
                        for k in range(KD):
                            wh_ = wbth2[k // 4]
                            wl_ = wbtl2[k // 4]
                            for lh, rh in ((wh_, vhp2), (wh_, vlp2), (wl_, vhp2)):
                                n += 1
                                nc.tensor.matmul(pp, lh[:, k % 4, ds], rh[:, k, :],
                                                 start=(n == 1), stop=(n == 3 * KD))
                        nc.vector.tensor_copy(php[:, dt, :], pp)
                        nc.vector.tensor_sub(plp[:, dt, :], pp, php[:, dt, :])
                    phl_pair[0] = (php, plp)
                ph = phl_pair[0][0].rearrange("p k n -> p k n")[:, :, po:po + NV]
                pl = phl_pair[0][1].rearrange("p k n -> p k n")[:, :, po:po + NV]

                # ---- C = tanh(Q P)  [NQ, NV]  f16 3-pass -> f16 ----
                c_ps = [psC.tile([128, NV], F32, tag="pc", name=f"pc{b}_{m}")
                        for m in range(MQ)]
                for k in range(KD):
                    for m in range(MQ):
                        ms = slice(m * 128, (m + 1) * 128)
                        for i, (lh, rh) in enumerate(
                                ((qth[:, k, ms], ph), (qth[:, k, ms], pl),
                                 (qtl[:, k, ms], ph))):
                            nc.tensor.matmul(c_ps[m], lh, rh[:, k, :],
                                             start=(k == 0 and i == 0),
                                             stop=(k == KD - 1 and i == 2))
                # ch padded to 256 free cols so DMA-transpose tiles are
                # 128-aligned; cols NV:256 are memset pad (never consumed).
                ch = mid.tile([128, MQ, 256], F16, tag="ch")
                for m in range(MQ):
                    nc.scalar.activation(ch[:, m, 0:NV], c_ps[m], AF.Tanh)
                    nc.vector.memset(ch[:, m, NV:256], 0.0)

                # ---- WqQT = Q W_q^T  [NQ, D]  f32r ----
                wqh = mid.tile([128, MQ, D], F16, tag="wqh")
                wql = mid.tile([128, MQ, D], F16, tag="wql")
                for m in range(MQ):
                    ms = slice(m * 128, (m + 1) * 128)
                    for h in range(2):
                        hs = slice(h * 512, (h + 1) * 512)
                        p = ps5.tile([128, 512], F32, tag="p5", name=f"pq{b}_{m}_{h}")
                        n = 0
                        for k in range(KD):
                            for lh, rh in ((qth, wqth), (qth, wqtl), (qtl, wqth)):
                                n += 1
                                nc.tensor.matmul(p, lh[:, k, ms], rh[:, k, hs],
                                                 start=(n == 1), stop=(n == 3 * KD))
                        nc.vector.tensor_copy(wqh[:, m, hs], p)
                        nc.vector.tensor_sub(wql[:, m, hs], p, wqh[:, m, hs])

                # ---- CT  [NV, NQ]  via DMA xbar transpose of C (off-PE) ----
                # mv=1 source tile spans cols 128:256 (68 real + 60 pad);
                # garbage lands in cth rows 68:128 which no consumer reads.
                cth = mid.tile([128, 2, NQ], F16, tag="cth")
                for mq in range(MQ):
                    for mv in range(2):
                        nc.sync.dma_start_transpose(
                            out=cth[:, mv, mq * 128:(mq + 1) * 128],
                            in_=ch[:, mq, mv * 128:(mv + 1) * 128])

                # ---- WvVT = V W_v^T  [NV, D]  f32r ----
                wvh = mid.tile([128, 2, D], F16, tag="wvh")
                wvl = mid.tile([128, 2, D], F16, tag="wvl")
                for mv in range(2):
                    rows = VROWS[mv]
                    vs = slice(mv * 128, mv * 128 + rows)
                    for h in range(2):
                        hs = slice(h * 512, (h + 1) * 512)
                        p = ps5.tile([128, 512], F32, tag="p5", name=f"pv{b}_{mv}_{h}")
                        n = 0
                        for k in range(KD):
                            for lh, rh in ((vth, wvth), (vth, wvtl), (vtl, wvth)):
                                n += 1
                                nc.tensor.matmul(p[:rows, :], lh[:, k, vs], rh[:, k, hs],
                                                 start=(n == 1), stop=(n == 3 * KD))
                        nc.vector.tensor_copy(wvh[:rows, mv, hs], p[:rows, :])
                        nc.vector.tensor_sub(wvl[:rows, mv, hs], p[:rows, :],
                                             wvh[:rows, mv, hs])

                # ---- H_vT = tanh(WvVT + C^T WqQT); DVE dot -> h_v chunks ----
                hvc = sm.tile([128, 2], F32, tag="hvc")
                hqc = sm.tile([128, MQ], F32, tag="hqc")
                scrd = sm.tile([128, D], F16, tag="scrd")
                hv_row = psR.tile([1, NV], F32, tag="prow_v", name=f"hvr{b}")
                hq_row = psR.tile([1, NQ], F32, tag="prow_q", name=f"hqr{b}")

                for mv in range(2):
                    rows = VROWS[mv]
                    vs = slice(mv * 128, mv * 128 + rows)
                    ht = hp.tile([128, D], F16, tag="h", name=f"hv{b}_{mv}")
                    # addend WvVT pre-written into PSUM by the Scalar engine
                    # (saves the identity-matmul columns on the PE); products
                    # then accumulate with start=False.
                    pv2 = [ps5.tile([128, 512], F32, tag="p5",
                                    name=f"phv{b}_{mv}_{h}") for h in range(2)]
                    for h in range(2):
                        hs = slice(h * 512, (h + 1) * 512)
                        nc.scalar.copy(pv2[h][:rows, :], wvh[:rows, mv, hs])
                    for mq in range(MQ):
                        for i, rh in enumerate((wqh, wql)):
                            for h in range(2):
                                hs = slice(h * 512, (h + 1) * 512)
                                nc.tensor.matmul(
                                    pv2[h][:rows, :], ch[:, mq, vs],
                                    rh[:, mq, hs], start=False,
                                    stop=(mq == MQ - 1 and i == 1),
                                    skip_group_check=True)
                    for h in range(2):
                        hs = slice(h * 512, (h + 1) * 512)
                        nc.scalar.activation(ht[:rows, hs], pv2[h][:rows, :],
                                             AF.Tanh)
                    nc.vector.scalar_tensor_tensor(
                        out=scrd[:rows, :], in0=ht[:rows, :], scalar=1.0,
                        in1=wvb[:rows, :], op0=ALU.mult, op1=ALU.mult,
                        accum_out=hvc[:rows, mv:mv + 1])
                    nc.tensor.matmul(hv_row[0:1, mv * 128:mv * 128 + rows],
                                     hvc[:rows, mv:mv + 1], identf[:rows, :rows],
                                     is_transpose=True,
                                     start=(mv == 0), stop=(mv == 1))

                # ---- H_qT = tanh(WqQT + CT^T WvVT); DVE dot -> h_q chunks ----
                for mq in range(MQ):
                    ms = slice(mq * 128, (mq + 1) * 128)
                    ht = hp.tile([128, D], F16, tag="h", name=f"hq{b}_{mq}")
                    pq2 = [ps5.tile([128, 512], F32, tag="p5",
                                    name=f"phq{b}_{mq}_{h}") for h in range(2)]
                    for h in range(2):
                        hs = slice(h * 512, (h + 1) * 512)
                        nc.scalar.copy(pq2[h], wqh[:, mq, hs])
                    for mv in range(2):
                        rows = VROWS[mv]
                        for i, rh in enumerate((wvh, wvl)):
                            for h in range(2):
                                hs = slice(h * 512, (h + 1) * 512)
                                nc.tensor.matmul(
                                    pq2[h], cth[:rows, mv, ms],
                                    rh[:rows, mv, hs], start=False,
                                    stop=(mv == 1 and i == 1),
                                    skip_group_check=True)
                    for h in range(2):
                        hs = slice(h * 512, (h + 1) * 512)
                        nc.scalar.activation(ht[:, hs], pq2[h], AF.Tanh)
                    nc.vector.scalar_tensor_tensor(
                        out=scrd, in0=ht, scalar=1.0, in1=wqb,
                        op0=ALU.mult, op1=ALU.mult,
                        accum_out=hqc[:, mq:mq + 1])
                    nc.tensor.matmul(hq_row[0:1, ms], hqc[:, mq:mq + 1], identf,
                                     is_transpose=True,
                                     start=(mq == 0), stop=(mq == MQ - 1))

                if dbg and b == 0:
                    nc.sync.dma_start(out=DBG["dHV"][:, :], in_=hvc)
                    nc.sync.dma_start(out=DBG["dHQ"][:, :], in_=hqc)


                def softmax_bcast(h_ps, n, tagp):
                    negm = sm.tile([1, 1], F32, tag=f"negm{tagp}")
                    nc.vector.reduce_max(negm, h_ps, axis=AX.X, negate=True)
                    ex = sm.tile([1, n], F16, tag=f"ex{tagp}")
                    ssum = sm.tile([1, 1], F32, tag=f"ssum{tagp}")
                    nc.scalar.activation(ex, h_ps, AF.Exp, bias=negm, accum_out=ssum)
                    rs = sm.tile([1, 1], F32, tag=f"rs{tagp}")
                    nc.vector.reciprocal(rs, ssum)
                    ones_s = sm.tile([1, 128], F16, tag=f"ones_s{tagp}")
                    nc.vector.tensor_scalar_mul(ones_s, ones_row, rs)
                    ab_ps = ps5.tile([128, n], F32, tag="p5", name=f"abps{tagp}{b}")
                    nc.tensor.matmul(ab_ps, ones_s, ex, start=True, stop=True)
                    ab = sm.tile([128, n], F16, tag=f"ab{tagp}")
                    nc.scalar.copy(ab, ab_ps)
                    return ab

                # ---- softmaxes + broadcast, then v_hat/q_hat on DVE ----
                av_b = softmax_bcast(hv_row, NV, "v")
                aq_b = softmax_bcast(hq_row, NQ, "q")
                vhat_sb = sm.tile([128, KD], F32, tag="vhat")
                qhat_sb = sm.tile([128, KD], F32, tag="qhat")
                for k in range(KD):
                    nc.vector.scalar_tensor_tensor(
                        out=scrd[:, 0:NV], in0=vth[:, k, :], scalar=1.0, in1=av_b,
                        op0=ALU.mult, op1=ALU.mult, accum_out=vhat_sb[:, k:k + 1])
                for k in range(KD):
                    nc.vector.scalar_tensor_tensor(
                        out=scrd[:, 0:NQ], in0=qth[:, k, :], scalar=1.0, in1=aq_b,
                        op0=ALU.mult, op1=ALU.mult, accum_out=qhat_sb[:, k:k + 1])
                nc.sync.dma_start(out=OV_d[b].rearrange("(k p) -> p k", p=128), in_=vhat_sb)
                nc.sync.dma_start(out=OQ_d[b].rearrange("(k p) -> p k", p=128), in_=qhat_sb)

    _dedup_ldweights(nc)
    nc.finalize()
    return nc


_BUILT = {}


def _split(x):
    hi = x.astype(np.float16)
    lo = (x - hi.astype(np.float32)).astype(np.float16)
    return np.ascontiguousarray(hi), np.ascontiguousarray(lo)


def kernel(V, Q, W_b, W_v, W_q, w_hv, w_hq, _trace=False):
    V = np.asarray(V, dtype=np.float32)
    Q = np.asarray(Q, dtype=np.float32)
    nb = B // NCORES
    QT = np.ascontiguousarray(Q.transpose(0, 2, 1))      # [B, D, NQ] f32
    VT = np.ascontiguousarray(V.transpose(0, 2, 1))      # [B, D, NV] f32
    QTh, QTl = _split(QT)
    VTh, VTl = _split(VT)
    WbTh, WbTl = _split(np.asarray(W_b, dtype=np.float32).T)
    WqTh, WqTl = _split(np.asarray(W_q, dtype=np.float32).T)
    WvTh, WvTl = _split(np.asarray(W_v, dtype=np.float32).T)
    wvb = np.ascontiguousarray(
        np.broadcast_to(np.asarray(w_hv, dtype=np.float32)[:, 0], (128, D)))
    wqb = np.ascontiguousarray(
        np.broadcast_to(np.asarray(w_hq, dtype=np.float32)[:, 0], (128, D)))

    if nb not in _BUILT:
        _BUILT[nb] = build(nb)
    nc = _BUILT[nb]

    in_maps = []
    for c in range(NCORES):
        sl = slice(c * nb, (c + 1) * nb)
        in_maps.append({
            "QTh": np.ascontiguousarray(QTh[sl]), "QTl": np.ascontiguousarray(QTl[sl]),
            "VTh": np.ascontiguousarray(VTh[sl]), "VTl": np.ascontiguousarray(VTl[sl]),
            "WbTh": WbTh, "WbTl": WbTl, "WqTh": WqTh, "WqTl": WqTl,
            "WvTh": WvTh, "WvTl": WvTl,
            "wvb": wvb, "wqb": wqb,
        })

    out = run_bass_kernel_spmd(nc, in_maps, core_ids=list(range(NCORES)),
                               trace=_trace)
    v_hat = np.concatenate([out.results[c]["OV"] for c in range(NCORES)], axis=0)
    q_hat = np.concatenate([out.results[c]["OQ"] for c in range(NCORES)], axis=0)
    if _trace:
        kernel._last_exec_ns = out.exec_time_ns
        kernel._last_results = out
    return (v_hat, q_hat)



# revision 34
# speedup vs baseline: 1.1644x; 1.1644x over previous
"""CoAttention forward on 8 TRN2 NeuronCores.

Data-parallel over batch B=64 (8 batches/core). Re-associated affinity:
C = tanh(Q @ (W_b @ V^T)) — 3.1e8 MACs/batch for the C-chain instead of
the reference association's 6.4e8. All heavy matmuls run as f16 hi/lo
3-pass (exact products on the PE; ~22-bit): P = W_b V^T (pair-packed,
two batches side by side in the 392-wide moving operand), C = Q P,
WqQT = Q W_q^T [NQ,D], WvVT = V W_v^T [NV,D]. The H matmuls run 2-pass
against single-f16 C (baseline-proven noise level):
H_vT = tanh(WvVT + C^T WqQT) [NV,D], H_qT = tanh(WqQT + CT^T WvVT) [NQ,D],
with the tanh addends pre-written into PSUM by the Scalar engine (the PE
is at its 1-col/cycle column roofline, so addend columns are moved off
it); the products then accumulate with start=False. CT comes from DMA
xbar transposes of (128-padded) C tiles rather than PE transposes, and a
post-legalize pass drops Ldweights whose stationary matches the previous
load (PE weights persist across matmuls).
(float32r was tried for the downstream matmuls — 3x fewer PE rows — but
its FP22 product rounding leaves 3e-3-rms noise on WqQT/WvVT which the
sqrt(512)-amplifying H matmuls turn into ~0.7 logit noise: tail batches
flip a top softmax logit and blow the 2e-2 gate.)
H stays [token, D] so the w_h dots run on the Vector engine; the
per-partition logit columns are PE-transposed into [1, n] psum rows for
the softmax + on-chip f16 broadcast. v_hat/q_hat are DVE dots against
the f16-hi inputs. Batch-0 inputs are DMA'd ahead of the big weight
tensors so the PE starts early.

kernel(**inputs) takes FULL inputs, shards internally, returns (v_hat, q_hat).
"""
import numpy as np

import concourse.bass as bass
import concourse.mybir as mybir
import concourse.tile as tile
from concourse import bacc
from concourse.bass_utils import run_bass_kernel_spmd
from concourse.masks import make_identity

AF = mybir.ActivationFunctionType
ALU = mybir.AluOpType
AX = mybir.AxisListType
F32 = mybir.dt.float32
F16 = mybir.dt.float16
F8E4 = mybir.dt.float8e4
F8E5 = mybir.dt.float8e5


def _dedup_ldweights(nc):
    """Drop Ldweights whose stationary AP matches the immediately preceding
    weight load with no intervening self-loading matmul: PE weights persist
    across matmuls (verified on HW), so the reload is pure overhead (~95ns
    per 128-row load, ~1519 elidable per core = ~136us). Ldweights carry no
    semaphore updates, so deletion is sem-safe; any waits are merged into
    the next instruction (compile() later re-legalizes multi-waits).
    Must run after TileContext exit (legalize done) and before finalize().
    """
    def sig(inst):
        ap = inst.ins[0]
        return (ap.memref, ap.offset, str(ap.ap), str(ap.dtype),
                str(inst.tile_size), str(inst.tile_position),
                str(inst.perf_mode), str(inst.is_transpose))

    ndrop = 0
    for b in nc.m.functions[0].blocks:
        insts = list(b.instructions)
        cur = None
        keep = []
        for inst in insts:
            if inst.opcode == 'Ldweights':
                si = inst.sync_info
                clean = si is None or (not len(si.on_wait)
                                       and not len(si.on_update))
                s = sig(inst)
                if s == cur and clean:
                    ndrop += 1
                    continue
                cur = s
            elif inst.opcode == 'Matmult' and inst.ldweights is None:
                cur = None  # self-loading matmul clobbers PE weights
            keep.append(inst)
        if len(keep) != len(insts):
            b.instructions[:] = keep
    return ndrop

B, NV, NQ, D = 64, 196, 512, 1024
NCORES = 8
NB = B // NCORES          # batches per core
KD = D // 128             # 8 feature k-tiles
MQ = NQ // 128            # 4 NQ tiles
NV1 = NV - 128            # 68 (second NV chunk)
VROWS = (128, NV1)


def build(nb=NB, dbg=False):
    nc = bacc.Bacc(None, target_bir_lowering=False)

    QTh_d = nc.dram_tensor("QTh", [nb, D, NQ], F16, kind="ExternalInput")
    QTl_d = nc.dram_tensor("QTl", [nb, D, NQ], F16, kind="ExternalInput")
    # fp8 operands for WqQT's correction passes (hi*lo and lo*hi, both
    # ~2^-11-scale terms) via DoubleRow. Scales: QTl8 = e4m3(QTl * 2^11),
    # WqTh8 = e5m2(WqT * 2^-11) (cancel in the product); QTh8 = e4m3(QTh),
    # WqTl8 = e5m2(WqTl) (already at target scale). Accumulate into the
    # same PSUM group as the f16 hi*hi pass.
    QTl8_d = nc.dram_tensor("QTl8", [nb, D, NQ], F8E4, kind="ExternalInput")
    QTh8_d = nc.dram_tensor("QTh8", [nb, D, NQ], F8E4, kind="ExternalInput")
    WqTh8_d = nc.dram_tensor("WqTh8", [D, D], F8E5, kind="ExternalInput")
    WqTl8_d = nc.dram_tensor("WqTl8", [D, D], F8E5, kind="ExternalInput")
    VTh_d = nc.dram_tensor("VTh", [nb, D, NV], F16, kind="ExternalInput")
    VTl_d = nc.dram_tensor("VTl", [nb, D, NV], F16, kind="ExternalInput")
    WbTh_d = nc.dram_tensor("WbTh", [D, D], F16, kind="ExternalInput")
    WbTl_d = nc.dram_tensor("WbTl", [D, D], F16, kind="ExternalInput")
    WqTh_d = nc.dram_tensor("WqTh", [D, D], F16, kind="ExternalInput")
    WvTh_d = nc.dram_tensor("WvTh", [D, D], F16, kind="ExternalInput")
    WvTl_d = nc.dram_tensor("WvTl", [D, D], F16, kind="ExternalInput")
    wvb_d = nc.dram_tensor("wvb", [128, D], F16, kind="ExternalInput")
    wqb_d = nc.dram_tensor("wqb", [128, D], F16, kind="ExternalInput")
    OV_d = nc.dram_tensor("OV", [nb, D], F32, kind="ExternalOutput")
    OQ_d = nc.dram_tensor("OQ", [nb, D], F32, kind="ExternalOutput")
    DBG = {}
    if dbg:
        for nm, shp in [("dHV", [128, 2]), ("dHQ", [128, MQ])]:
            DBG[nm] = nc.dram_tensor(nm, shp, F32, kind="ExternalOutput")
        for nm, shp in [("dCh", [128, MQ, NV]), ("dCl", [128, MQ, NV]),
                        ("dWQh", [128, MQ, D]), ("dWQl", [128, MQ, D]),
                        ("dWVh", [128, 2, D]), ("dWVl", [128, 2, D]),
                        ("dHT", [128, D]), ("dPh", [128, KD, NV]),
                        ("dPl", [128, KD, NV])]:
            DBG[nm] = nc.dram_tensor(nm, shp, F16, kind="ExternalOutput")

    with tile.TileContext(nc) as tc:
        with (
            tc.tile_pool(name="wsb", bufs=1) as wsb,
            tc.tile_pool(name="qthp", bufs=2) as qthp,
            tc.tile_pool(name="vthp", bufs=2) as vthp,
            tc.tile_pool(name="io1", bufs=1) as io1,
            tc.tile_pool(name="mid", bufs=1) as mid,
            tc.tile_pool(name="hp", bufs=1) as hp,
            tc.tile_pool(name="sm", bufs=1) as sm,
            tc.tile_pool(name="psC", bufs=4, space="PSUM") as psC,
            tc.tile_pool(name="ps5", bufs=2, space="PSUM") as ps5,
        ):
            # ---- persistent weights ----
            wbth2 = []
            wbtl2 = []
            for hi, kh in enumerate((slice(0, 4), slice(4, KD))):
                th = wsb.tile([128, 4, D], F16, name=f"wbth{hi}")
                nc.sync.dma_start(out=th,
                                  in_=WbTh_d.rearrange("(k p) d -> p k d", p=128)[:, kh, :])
                wbth2.append(th)
                tl = wsb.tile([128, 4, D], F16, name=f"wbtl{hi}")
                nc.sync.dma_start(out=tl,
                                  in_=WbTl_d.rearrange("(k p) d -> p k d", p=128)[:, kh, :])
                wbtl2.append(tl)
            # batch-0 inputs first so P can start while big weights stream
            pre_in = {}
            for _b in (0, 1):
                pv = vthp.tile([128, KD, 2, NV], F16, tag="vthp", name=f"vthp{_b}")
                pl_ = vthp.tile([128, KD, 2, NV], F16, tag="vtlp", name=f"vtlp{_b}")
                for kh in ((slice(0, 4), slice(4, KD)) if _b == 0 else (slice(0, KD),)):
                    for t in (0, 1):
                        nc.sync.dma_start(
                            out=pv[:, kh, t, :],
                            in_=VTh_d[2 * _b + t].rearrange("(k p) n -> p k n", p=128)[:, kh, :])
                        nc.sync.dma_start(
                            out=pl_[:, kh, t, :],
                            in_=VTl_d[2 * _b + t].rearrange("(k p) n -> p k n", p=128)[:, kh, :])
                pre_in[_b] = (pv, pl_)
                if _b == 0:
                    q0 = qthp.tile([128, KD, NQ], F16, tag="qth", name="qth0")
                    nc.sync.dma_start(out=q0, in_=QTh_d[0].rearrange("(k p) n -> p k n", p=128))
                    q0l = io1.tile([128, KD, NQ], F16, tag="qtl", name="qtl0")
                    nc.sync.dma_start(out=q0l, in_=QTl_d[0].rearrange("(k p) n -> p k n", p=128))
                    q08 = io1.tile([128, KD, NQ], F8E4, tag="qtl8", name="qtl8_0")
                    nc.sync.dma_start(out=q08, in_=QTl8_d[0].rearrange("(k p) n -> p k n", p=128))
                    q0h8 = io1.tile([128, KD, NQ], F8E4, tag="qth8", name="qth8_0")
                    nc.sync.dma_start(out=q0h8, in_=QTh8_d[0].rearrange("(k p) n -> p k n", p=128))
                    pre_in["q0"] = (q0, q0l, q08, q0h8)
            wqth = wsb.tile([128, KD, D], F16, name="wqth")
            nc.sync.dma_start(out=wqth, in_=WqTh_d.rearrange("(k p) d -> p k d", p=128))
            wql8 = wsb.tile([128, KD, D], F8E5, name="wql8")
            nc.sync.dma_start(out=wql8, in_=WqTl8_d.rearrange("(k p) d -> p k d", p=128))
            wvth = wsb.tile([128, KD, D], F16, name="wvth")
            nc.sync.dma_start(out=wvth, in_=WvTh_d.rearrange("(k p) d -> p k d", p=128))
            wvtl = wsb.tile([128, KD, D], F16, name="wvtl")
            nc.sync.dma_start(out=wvtl, in_=WvTl_d.rearrange("(k p) d -> p k d", p=128))
            wq8 = wsb.tile([128, KD, D], F8E5, name="wq8")
            nc.sync.dma_start(out=wq8, in_=WqTh8_d.rearrange("(k p) d -> p k d", p=128))
            wvb = wsb.tile([128, D], F16, name="wvb")
            nc.sync.dma_start(out=wvb, in_=wvb_d[:, :])
            wqb = wsb.tile([128, D], F16, name="wqb")
            nc.sync.dma_start(out=wqb, in_=wqb_d[:, :])
            identf = wsb.tile([128, 128], F32, name="identf")
            make_identity(nc, identf)
            ones_row = wsb.tile([1, 128], F32)
            nc.vector.memset(ones_row, 1.0)

            phl_pair = [None]
            for b in range(nb):
                pi = b % 2
                po = pi * NV
                if pi == 0:
                    pair = b // 2
                    if pair in pre_in:
                        vthp_t, vtlp_t = pre_in.pop(pair)
                    else:
                        vthp_t = vthp.tile([128, KD, 2, NV], F16, tag="vthp",
                                           name=f"vthp{pair}")
                        nc.sync.dma_start(out=vthp_t[:, :, 0, :],
                                          in_=VTh_d[b].rearrange("(k p) n -> p k n", p=128))
                        nc.sync.dma_start(out=vthp_t[:, :, 1, :],
                                          in_=VTh_d[b + 1].rearrange("(k p) n -> p k n", p=128))
                        vtlp_t = vthp.tile([128, KD, 2, NV], F16, tag="vtlp",
                                           name=f"vtlp{pair}")
                        nc.sync.dma_start(out=vtlp_t[:, :, 0, :],
                                          in_=VTl_d[b].rearrange("(k p) n -> p k n", p=128))
                        nc.sync.dma_start(out=vtlp_t[:, :, 1, :],
                                          in_=VTl_d[b + 1].rearrange("(k p) n -> p k n", p=128))
                vth = vthp_t.rearrange("p k t n -> p k (t n)")[:, :, po:po + NV]
                vtl = vtlp_t.rearrange("p k t n -> p k (t n)")[:, :, po:po + NV]
                if b == 0 and "q0" in pre_in:
                    qth, qtl, qtl8, qth8 = pre_in.pop("q0")
                else:
                    qth = qthp.tile([128, KD, NQ], F16, tag="qth", name=f"qth{b}")
                    nc.sync.dma_start(out=qth, in_=QTh_d[b].rearrange("(k p) n -> p k n", p=128))
                    qtl = io1.tile([128, KD, NQ], F16, tag="qtl", name=f"qtl{b}")
                    nc.sync.dma_start(out=qtl, in_=QTl_d[b].rearrange("(k p) n -> p k n", p=128))
                    qtl8 = io1.tile([128, KD, NQ], F8E4, tag="qtl8", name=f"qtl8_{b}")
                    nc.sync.dma_start(out=qtl8, in_=QTl8_d[b].rearrange("(k p) n -> p k n", p=128))
                    qth8 = io1.tile([128, KD, NQ], F8E4, tag="qth8", name=f"qth8_{b}")
                    nc.sync.dma_start(out=qth8, in_=QTh8_d[b].rearrange("(k p) n -> p k n", p=128))


                # ---- P = W_b V^T  [D, 2*NV]  f16 3-pass, pair-packed ----
                if pi == 0:
                    vhp2 = vthp_t.rearrange("p k t n -> p k (t n)")
                    vlp2 = vtlp_t.rearrange("p k t n -> p k (t n)")
                    php = mid.tile([128, KD, 2 * NV], F16, tag="php")
                    plp = mid.tile([128, KD, 2 * NV], F16, tag="plp")
                    for dt in range(KD):
                        ds = slice(dt * 128, (dt + 1) * 128)
                        pp = psC.tile([128, 2 * NV], F32, tag="pc", name=f"pp{b}_{dt}")
                        n = 0
                        for k in range(KD):
                            wh_ = wbth2[k // 4]
                            wl_ = wbtl2[k // 4]
                            for lh, rh in ((wh_, vhp2), (wh_, vlp2), (wl_, vhp2)):
                                n += 1
                                nc.tensor.matmul(pp, lh[:, k % 4, ds], rh[:, k, :],
                                                 start=(n == 1), stop=(n == 3 * KD))
                        nc.vector.tensor_copy(php[:, dt, :], pp)
                        nc.vector.tensor_sub(plp[:, dt, :], pp, php[:, dt, :])
                    phl_pair[0] = (php, plp)
                ph = phl_pair[0][0].rearrange("p k n -> p k n")[:, :, po:po + NV]
                pl = phl_pair[0][1].rearrange("p k n -> p k n")[:, :, po:po + NV]

                # ---- C = tanh(Q P)  [NQ, NV]  f16 3-pass -> f16 ----
                c_ps = [psC.tile([128, NV], F32, tag="pc", name=f"pc{b}_{m}")
                        for m in range(MQ)]
                for k in range(KD):
                    for m in range(MQ):
                        ms = slice(m * 128, (m + 1) * 128)
                        for i, (lh, rh) in enumerate(
                                ((qth[:, k, ms], ph), (qth[:, k, ms], pl),
                                 (qtl[:, k, ms], ph))):
                            nc.tensor.matmul(c_ps[m], lh, rh[:, k, :],
                                             start=(k == 0 and i == 0),
                                             stop=(k == KD - 1 and i == 2))
                # ch padded to 256 free cols so DMA-transpose tiles are
                # 128-aligned; cols NV:256 are memset pad (never consumed).
                ch = mid.tile([128, MQ, 256], F16, tag="ch")
                for m in range(MQ):
                    nc.scalar.activation(ch[:, m, 0:NV], c_ps[m], AF.Tanh)
                    nc.vector.memset(ch[:, m, NV:256], 0.0)

                # ---- WqQT = Q W_q^T  [NQ, D] ----
                # pass 1 (hi*hi) in f16; both ~2^-11 correction passes
                # (hi*lo, lo*hi) as fp8 DoubleRow, 2 k-planes per matmul,
                # accumulating into the same PSUM group (scales cancel).
                wqh = mid.tile([128, MQ, D], F16, tag="wqh")
                wql = mid.tile([128, MQ, D], F16, tag="wql")
                for m in range(MQ):
                    ms = slice(m * 128, (m + 1) * 128)
                    pb = [ps5.tile([128, 512], F32, tag="p5", name=f"pq{b}_{m}_{h}")
                          for h in range(2)]
                    for k in range(KD):
                        for h in range(2):
                            hs = slice(h * 512, (h + 1) * 512)
                            nc.tensor.matmul(pb[h], qth[:, k, ms], wqth[:, k, hs],
                                             start=(k == 0), stop=False)
                    for lh8, rh8 in ((qth8, wql8), (qtl8, wq8)):
                        last8 = rh8 is wq8
                        for j in range(KD // 2):
                            ks = slice(2 * j, 2 * j + 2)
                            for h in range(2):
                                hs = slice(h * 512, (h + 1) * 512)
                                nc.tensor.matmul(pb[h], lh8[:, ks, ms],
                                                 rh8[:, ks, hs],
                                                 perf_mode=mybir.MatmulPerfMode.DoubleRow,
                                                 start=False,
                                                 stop=(last8 and j == KD // 2 - 1),
                                                 skip_group_check=True)
                    for h in range(2):
                        hs = slice(h * 512, (h + 1) * 512)
                        nc.vector.tensor_copy(wqh[:, m, hs], pb[h])
                        nc.vector.tensor_sub(wql[:, m, hs], pb[h], wqh[:, m, hs])

                # ---- CT  [NV, NQ]  via DMA xbar transpose of C (off-PE) ----
                # mv=1 source tile spans cols 128:256 (68 real + 60 pad);
                # garbage lands in cth rows 68:128 which no consumer reads.
                cth = mid.tile([128, 2, NQ], F16, tag="cth")
                for mq in range(MQ):
                    for mv in range(2):
                        nc.sync.dma_start_transpose(
                            out=cth[:, mv, mq * 128:(mq + 1) * 128],
                            in_=ch[:, mq, mv * 128:(mv + 1) * 128])

                # ---- WvVT = V W_v^T  [NV, D]  f32r ----
                wvh = mid.tile([128, 2, D], F16, tag="wvh")
                wvl = mid.tile([128, 2, D], F16, tag="wvl")
                for mv in range(2):
                    rows = VROWS[mv]
                    vs = slice(mv * 128, mv * 128 + rows)
                    for h in range(2):
                        hs = slice(h * 512, (h + 1) * 512)
                        p = ps5.tile([128, 512], F32, tag="p5", name=f"pv{b}_{mv}_{h}")
                        n = 0
                        for k in range(KD):
                            for lh, rh in ((vth, wvth), (vth, wvtl), (vtl, wvth)):
                                n += 1
                                nc.tensor.matmul(p[:rows, :], lh[:, k, vs], rh[:, k, hs],
                                                 start=(n == 1), stop=(n == 3 * KD))
                        nc.vector.tensor_copy(wvh[:rows, mv, hs], p[:rows, :])
                        nc.vector.tensor_sub(wvl[:rows, mv, hs], p[:rows, :],
                                             wvh[:rows, mv, hs])

                # ---- H_vT = tanh(WvVT + C^T WqQT); DVE dot -> h_v chunks ----
                hvc = sm.tile([128, 2], F32, tag="hvc")
                hqc = sm.tile([128, MQ], F32, tag="hqc")
                scrd = sm.tile([128, D], F16, tag="scrd")
                hv_row = ps5.tile([1, NV], F32, tag="prow", name=f"hvr{b}")
                hq_row = ps5.tile([1, NQ], F32, tag="prow", name=f"hqr{b}")

                for mv in range(2):
                    rows = VROWS[mv]
                    vs = slice(mv * 128, mv * 128 + rows)
                    ht = hp.tile([128, D], F16, tag="h", name=f"hv{b}_{mv}")
                    # addend WvVT pre-written into PSUM by the Scalar engine
                    # (saves the identity-matmul columns on the PE); products
                    # then accumulate with start=False.
                    pv2 = [ps5.tile([128, 512], F32, tag="p5",
                                    name=f"phv{b}_{mv}_{h}") for h in range(2)]
                    for h in range(2):
                        hs = slice(h * 512, (h + 1) * 512)
                        nc.scalar.copy(pv2[h][:rows, :], wvh[:rows, mv, hs])
                    for mq in range(MQ):
                        for i, rh in enumerate((wqh, wql)):
                            for h in range(2):
                                hs = slice(h * 512, (h + 1) * 512)
                                nc.tensor.matmul(
                                    pv2[h][:rows, :], ch[:, mq, vs],
                                    rh[:, mq, hs], start=False,
                                    stop=(mq == MQ - 1 and i == 1),
                                    skip_group_check=True)
                    for h in range(2):
                        hs = slice(h * 512, (h + 1) * 512)
                        nc.scalar.activation(ht[:rows, hs], pv2[h][:rows, :],
                                             AF.Tanh)
                    nc.vector.scalar_tensor_tensor(
                        out=scrd[:rows, :], in0=ht[:rows, :], scalar=1.0,
                        in1=wvb[:rows, :], op0=ALU.mult, op1=ALU.mult,
                        accum_out=hvc[:rows, mv:mv + 1])
                    nc.tensor.matmul(hv_row[0:1, mv * 128:mv * 128 + rows],
                                     hvc[:rows, mv:mv + 1], identf[:rows, :rows],
                                     is_transpose=True,
                                     start=(mv == 0), stop=(mv == 1))

                # ---- H_qT = tanh(WqQT + CT^T WvVT); DVE dot -> h_q chunks ----
                for mq in range(MQ):
                    ms = slice(mq * 128, (mq + 1) * 128)
                    ht = hp.tile([128, D], F16, tag="h", name=f"hq{b}_{mq}")
                    pq2 = [ps5.tile([128, 512], F32, tag="p5",
                                    name=f"phq{b}_{mq}_{h}") for h in range(2)]
                    for h in range(2):
                        hs = slice(h * 512, (h + 1) * 512)
                        nc.scalar.copy(pq2[h], wqh[:, mq, hs])
                    for mv in range(2):
                        rows = VROWS[mv]
                        for i, rh in enumerate((wvh, wvl)):
                            for h in range(2):
                                hs = slice(h * 512, (h + 1) * 512)
                                nc.tensor.matmul(
                                    pq2[h], cth[:rows, mv, ms],
                                    rh[:rows, mv, hs], start=False,
                                    stop=(mv == 1 and i == 1),
                                    skip_group_check=True)
                    for h in range(2):
                        hs = slice(h * 512, (h + 1) * 512)
                        nc.scalar.activation(ht[:, hs], pq2[h], AF.Tanh)
                    nc.vector.scalar_tensor_tensor(
                        out=scrd, in0=ht, scalar=1.0, in1=wqb,
                        op0=ALU.mult, op1=ALU.mult,
                        accum_out=hqc[:, mq:mq + 1])
                    nc.tensor.matmul(hq_row[0:1, ms], hqc[:, mq:mq + 1], identf,
                                     is_transpose=True,
                                     start=(mq == 0), stop=(mq == MQ - 1))

                if dbg and b == 0:
                    nc.sync.dma_start(out=DBG["dHV"][:, :], in_=hvc)
                    nc.sync.dma_start(out=DBG["dHQ"][:, :], in_=hqc)


                def softmax_bcast(h_ps, n, tagp):
                    negm = sm.tile([1, 1], F32, tag=f"negm{tagp}")
                    nc.vector.reduce_max(negm, h_ps, axis=AX.X, negate=True)
                    ex = sm.tile([1, n], F16, tag=f"ex{tagp}")
                    ssum = sm.tile([1, 1], F32, tag=f"ssum{tagp}")
                    nc.scalar.activation(ex, h_ps, AF.Exp, bias=negm, accum_out=ssum)
                    rs = sm.tile([1, 1], F32, tag=f"rs{tagp}")
                    nc.vector.reciprocal(rs, ssum)
                    ones_s = sm.tile([1, 128], F16, tag=f"ones_s{tagp}")
                    nc.vector.tensor_scalar_mul(ones_s, ones_row, rs)
                    ab_ps = ps5.tile([128, n], F32, tag="p5", name=f"abps{tagp}{b}")
                    nc.tensor.matmul(ab_ps, ones_s, ex, start=True, stop=True)
                    ab = sm.tile([128, n], F16, tag=f"ab{tagp}")
                    nc.scalar.copy(ab, ab_ps)
                    return ab

                # ---- softmaxes + broadcast, then v_hat/q_hat on DVE ----
                av_b = softmax_bcast(hv_row, NV, "v")
                aq_b = softmax_bcast(hq_row, NQ, "q")
                vhat_sb = sm.tile([128, KD], F32, tag="vhat")
                qhat_sb = sm.tile([128, KD], F32, tag="qhat")
                for k in range(KD):
                    nc.vector.scalar_tensor_tensor(
                        out=scrd[:, 0:NV], in0=vth[:, k, :], scalar=1.0, in1=av_b,
                        op0=ALU.mult, op1=ALU.mult, accum_out=vhat_sb[:, k:k + 1])
                for k in range(KD):
                    nc.vector.scalar_tensor_tensor(
                        out=scrd[:, 0:NQ], in0=qth[:, k, :], scalar=1.0, in1=aq_b,
                        op0=ALU.mult, op1=ALU.mult, accum_out=qhat_sb[:, k:k + 1])
                nc.sync.dma_start(out=OV_d[b].rearrange("(k p) -> p k", p=128), in_=vhat_sb)
                nc.sync.dma_start(out=OQ_d[b].rearrange("(k p) -> p k", p=128), in_=qhat_sb)

    _dedup_ldweights(nc)
    nc.finalize()
    return nc


_BUILT = {}


def _split(x):
    hi = x.astype(np.float16)
    lo = (x - hi.astype(np.float32)).astype(np.float16)
    return np.ascontiguousarray(hi), np.ascontiguousarray(lo)


def kernel(V, Q, W_b, W_v, W_q, w_hv, w_hq, _trace=False):
    V = np.asarray(V, dtype=np.float32)
    Q = np.asarray(Q, dtype=np.float32)
    nb = B // NCORES
    QT = np.ascontiguousarray(Q.transpose(0, 2, 1))      # [B, D, NQ] f32
    VT = np.ascontiguousarray(V.transpose(0, 2, 1))      # [B, D, NV] f32
    QTh, QTl = _split(QT)
    VTh, VTl = _split(VT)
    WbTh, WbTl = _split(np.asarray(W_b, dtype=np.float32).T)
    WqTh, WqTl = _split(np.asarray(W_q, dtype=np.float32).T)
    WvTh, WvTl = _split(np.asarray(W_v, dtype=np.float32).T)
    import ml_dtypes
    QTl8 = np.ascontiguousarray(
        (QTl.astype(np.float32) * 2048.0).astype(ml_dtypes.float8_e4m3fn))
    QTh8 = np.ascontiguousarray(QTh.astype(ml_dtypes.float8_e4m3fn))
    WqTh8 = np.ascontiguousarray(
        (WqTh.astype(np.float32) * (1.0 / 2048.0))
        .astype(ml_dtypes.float8_e5m2))
    WqTl8 = np.ascontiguousarray(WqTl.astype(ml_dtypes.float8_e5m2))
    wvb = np.ascontiguousarray(
        np.broadcast_to(np.asarray(w_hv, dtype=np.float32)[:, 0],
                        (128, D))).astype(np.float16)
    wqb = np.ascontiguousarray(
        np.broadcast_to(np.asarray(w_hq, dtype=np.float32)[:, 0],
                        (128, D))).astype(np.float16)

    if nb not in _BUILT:
        _BUILT[nb] = build(nb)
    nc = _BUILT[nb]

    in_maps = []
    for c in range(NCORES):
        sl = slice(c * nb, (c + 1) * nb)
        in_maps.append({
            "QTh": np.ascontiguousarray(QTh[sl]), "QTl": np.ascontiguousarray(QTl[sl]),
            "QTl8": np.ascontiguousarray(QTl8[sl]),
            "QTh8": np.ascontiguousarray(QTh8[sl]),
            "VTh": np.ascontiguousarray(VTh[sl]), "VTl": np.ascontiguousarray(VTl[sl]),
            "WbTh": WbTh, "WbTl": WbTl, "WqTh": WqTh,
            "WvTh": WvTh, "WvTl": WvTl,
            "WqTh8": WqTh8, "WqTl8": WqTl8,
            "wvb": wvb, "wqb": wqb,
        })

    out = run_bass_kernel_spmd(nc, in_maps, core_ids=list(range(NCORES)),
                               trace=_trace)
    v_hat = np.concatenate([out.results[c]["OV"] for c in range(NCORES)], axis=0)
    q_hat = np.concatenate([out.results[c]["OQ"] for c in range(NCORES)], axis=0)
    if _trace:
        kernel._last_exec_ns = out.exec_time_ns
        kernel._last_results = out
    return (v_hat, q_hat)

